# revision 13
# baseline (speedup 1.0000x reference)
"""Additive (Bahdanau) attention on 8 TRN2 NeuronCores — separable-score
formulation.

Math per batch b:
    q = queries[b] @ W_q                  (Q, H)
    k = keys[b]    @ W_k                  (K, H)
    S[i, j] = sum_h w_v[h] * tanh(q[i,h] + k[j,h])
    out[b]  = softmax_j(S masked) @ values[b]

Instead of materializing the (Q, K, H) tanh tensor (elementwise-engine
bound), tanh is approximated by a regularized Fourier-extension series

    tanh(u) ~= c1*u + sum_{r=1..R} b_r sin(r*w1*u),   w1 = pi/PERIOD

whose angle-addition expansion is separable in (q, k):

    S[i,j] ~= sum_h sum_r (w_h b_r) [sin(r w1 q) cos(r w1 k)
                                     + cos(r w1 q) sin(r w1 k)]
              + c1 * sum_h w_h k_jh   (+ a per-row term softmax discards)

so S becomes R chained 128-contraction matmuls over trig features, and the
linear term folds into the exp bias.  The HW Sin table is only valid for
|arg| <= pi, so just sin/cos(w1 q) are computed on ACT (|w1 q| + pi/2 < pi)
and higher harmonics come from the Chebyshev angle ladder on DVE in fp16:

    X_r = 2 cos(w1 x) * X_{r-1} - X_{r-2}

which runs the sin and cos chains together in one (128, n) tile
(partitions = 64 h x {sin, cos}).  Rel err of the whole approximation
(incl. fp16 features) ~= 5e-3 vs the exact reference.

Sharding: the prefix mask means only sum(valid_lens) key columns carry
work.  Valid keys are split into <=128-key jobs; each of the 8 cores runs
4 job slots fed by two query feature sets (set0 serves slots 0-2, set1
serves slot 3; a set is one (batch, q-half) of 512 queries).  Each job
emits unnormalized partials (O^T = V^T P, l = 1^T P); the host sums
partials per batch and divides.
"""

import sys

sys.path.insert(0, "/opt/trn_rl_repo")

import numpy as np

B, Q, KLEN, D_IN, H, D_V = 4, 1024, 1024, 256, 64, 128
NCORES = 8
NSLOT = 4  # job slots per core: slots 0-2 -> set0, slot 3 -> set1
MASK_VAL = -1.0e6

R = 12  # Fourier harmonics
PERIOD = 13.0
OM1 = np.pi / PERIOD
UFIT = 9.8
LAM = 0.1

_CACHE = {}
_COEF = None
LAST_RESULT = None


def _fit_coeffs():
    """Host-side (weights-only) fit: tanh(u) ~= c1*u + sum b_r sin(r w1 u)."""
    global _COEF
    if _COEF is not None:
        return _COEF
    u = np.linspace(-UFIT, UFIT, 6001)
    w = 0.25 + np.exp(-(u**2) / (2 * 3.0**2))
    cols = [u / UFIT] + [np.sin((r + 1) * np.pi * u / PERIOD) for r in range(R)]
    A = np.stack(cols, axis=1)
    AtA = (A * w[:, None] ** 2).T @ A + LAM * np.eye(A.shape[1])
    Aty = (A * w[:, None] ** 2).T @ np.tanh(u)
    coef = np.linalg.solve(AtA, Aty)
    _COEF = (coef[0] / UFIT, coef[1:].copy())
    return _COEF


def _plan(vl):
    """Split each batch's valid-key prefix into <=128-key jobs per q-half and
    pack them into 8 cores x (one 3-job set0 + one 1-job set1).

    Returns cores: list of 8 entries, each a list of NSLOT jobs
    (b, qh, start, cnt) with cnt == 0 for padding slots."""
    combos = []  # ((b, qh), [(start, cnt), ...])
    for b, v in enumerate(vl):
        nb = -(-v // 128) if v > 0 else 0
        chunks = []
        s = 0
        for i in range(nb):
            cnt = min(128, v - s)
            chunks.append((s, cnt))
            s += cnt
        for qh in range(2):
            if chunks:
                combos.append(((b, qh), list(chunks)))
    assert sum(len(c[1]) for c in combos) <= 4 * NCORES, (
        "job count exceeds slot capacity; vl sum too large for this plan"
    )
    # Greedy: fill 8 triple-slots (set0) and 8 single-slots (set1).
    triples = []  # (combo, [jobs up to 3])
    singles = []  # (combo, job)
    combos = sorted(combos, key=lambda c: -len(c[1]))
    leftovers = []
    for key, chunks in combos:
        i = 0
        while len(chunks) - i >= 3 and len(triples) < NCORES:
            triples.append((key, chunks[i : i + 3]))
            i += 3
        leftovers.append((key, chunks[i:]))
    # place leftovers: prefer singles; group of 2-3 can take a triple slot
    for key, chunks in sorted(leftovers, key=lambda c: -len(c[1])):
        i = 0
        while len(chunks) - i >= 2 and len(triples) < NCORES:
            take = min(3, len(chunks) - i)
            triples.append((key, chunks[i : i + take]))
            i += take
        while i < len(chunks):
            if len(singles) < NCORES:
                singles.append((key, chunks[i]))
            elif len(triples) < NCORES:
                triples.append((key, [chunks[i]]))
            else:
                raise AssertionError("packing failed")
            i += 1
    while len(triples) < NCORES:
        triples.append(((0, 0), []))
    while len(singles) < NCORES:
        singles.append(((0, 0), None))
    cores = []
    for c in range(NCORES):
        (b0, qh0), t_jobs = triples[c]
        (b1, qh1), s_job = singles[c]
        jobs = []
        for i in range(3):
            if i < len(t_jobs):
                jobs.append((b0, qh0, t_jobs[i][0], t_jobs[i][1]))
            else:
                jobs.append((b0, qh0, 0, 0))
        if s_job is not None:
            jobs.append((b1, qh1, s_job[0], s_job[1]))
        else:
            jobs.append((b1, qh1, 0, 0))
        cores.append(jobs)
    return cores


def _build(repeat=1):
    import concourse.tile as tile
    from concourse import bacc, mybir

    fp32 = mybir.dt.float32
    fp16 = mybir.dt.float16
    Sin = mybir.ActivationFunctionType.Sin
    Exp = mybir.ActivationFunctionType.Exp
    Copy = mybir.ActivationFunctionType.Copy
    mult = mybir.AluOpType.mult
    sub = mybir.AluOpType.subtract
    add = mybir.AluOpType.add

    nc = bacc.Bacc(
        "TRN2", target_bir_lowering=False, debug=False, num_devices=NCORES
    )
    qtsE = nc.dram_tensor("qts", [128, 2048], fp16, kind="ExternalInput").ap()
    ktsE = nc.dram_tensor("kts", [128, 1024], fp16, kind="ExternalInput").ap()
    # vtsA: per job 65 cols = [V[:, 0:64] | ones]; the ones column makes the
    # O-matmul emit l (= 1^T P) as output row 64.  vtsB: V[:, 64:128].
    vtsAE = nc.dram_tensor("vtsA", [128, NSLOT * 65], fp16, kind="ExternalInput").ap()
    vtsBE = nc.dram_tensor("vtsB", [128, NSLOT * 64], fp16, kind="ExternalInput").ap()
    maskE = nc.dram_tensor("mask", [128, 4], fp32, kind="ExternalInput").ap()
    wq2E = nc.dram_tensor("wq2", [128, 256], fp16, kind="ExternalInput").ap()
    wk2E = nc.dram_tensor("wk2", [128, 256], fp16, kind="ExternalInput").ap()
    wvecE = nc.dram_tensor("wvec", [128, 1], fp16, kind="ExternalInput").ap()
    wbsE = nc.dram_tensor("wbs", [128, R], fp32, kind="ExternalInput").ap()
    hpE = nc.dram_tensor("hp", [128, 3], fp32, kind="ExternalInput").ap()
    outAE = nc.dram_tensor("outA", [65, NSLOT * 512], fp32, kind="ExternalOutput").ap()
    outBE = nc.dram_tensor("outB", [64, NSLOT * 512], fp32, kind="ExternalOutput").ap()

    with tile.TileContext(nc) as tc:
        with (
            tc.tile_pool(name="const", bufs=1) as cp,
            tc.tile_pool(name="feat", bufs=2) as fp,
            tc.tile_pool(name="ps", bufs=1, space="PSUM") as psp,
        ):
            # ---- input DMAs (q-path first), spread across rings
            qts = cp.tile([128, 2048], fp16)
            nc.sync.dma_start(qts[:, 0:1024], qtsE[:, 0:1024])
            wq2 = cp.tile([128, 256], fp16)
            nc.scalar.dma_start(wq2[:], wq2E[:, :])
            nc.sync.dma_start(qts[:, 1024:2048], qtsE[:, 1024:2048])
            kts = cp.tile([128, 1024], fp16)
            nc.scalar.dma_start(kts[:], ktsE[:, :])
            wk2 = cp.tile([128, 256], fp16)
            nc.scalar.dma_start(wk2[:], wk2E[:, :])
            hp = cp.tile([128, 3], fp32)
            nc.sync.dma_start(hp[:], hpE[:, :])
            wbs = cp.tile([128, R], fp32)
            nc.sync.dma_start(wbs[:], wbsE[:, :])
            vtsA = cp.tile([128, NSLOT * 65], fp16)
            nc.gpsimd.dma_start(vtsA[:], vtsAE[:, :])
            vtsB = cp.tile([128, NSLOT * 64], fp16)
            nc.gpsimd.dma_start(vtsB[:], vtsBE[:, :])
            mask_sb = cp.tile([128, 4], fp32)
            nc.sync.dma_start(mask_sb[:], maskE[:, :])
            wvec = cp.tile([128, 1], fp16)
            nc.scalar.dma_start(wvec[:], wvecE[:, :])

            # ladder r=0 tiles: q-side [sin0; cos0] = [0; 1], k-side [cos0; sin0]
            x0q = cp.tile([128, 1024], fp16, name="x0q")
            nc.gpsimd.memset(x0q[0:64, :], 0.0)
            nc.gpsimd.memset(x0q[64:128, :], 1.0)
            x0k = cp.tile([128, 512], fp16, name="x0k")
            nc.gpsimd.memset(x0k[0:64, :], 1.0)
            nc.gpsimd.memset(x0k[64:128, :], 0.0)

            for rep in range(repeat):
                sfx = f"_{rep}"
                # ---- projections (stationaries duplicated so psum has both
                # halves: rows 0:64 and 64:128 hold identical h-values)
                qp_ps = [
                    psp.tile([128, 512], fp32, tag=f"qp{s}", name=f"qp{s}{sfx}")
                    for s in range(2)
                ]
                for s in range(2):
                    for c in range(2):
                        nc.tensor.matmul(
                            qp_ps[s][:],
                            wq2[:, c * 128 : (c + 1) * 128],
                            qts[:, s * 1024 + c * 512 : s * 1024 + (c + 1) * 512],
                            start=(c == 0),
                            stop=(c == 1),
                        )
                kp_ps = psp.tile([128, 512], fp32, tag="kp", name=f"kp{sfx}")
                for c in range(2):
                    nc.tensor.matmul(
                        kp_ps[:],
                        wk2[:, c * 128 : (c + 1) * 128],
                        kts[:, c * 512 : (c + 1) * 512],
                        start=(c == 0),
                        stop=(c == 1),
                    )

                # ---- ACT anchors (Sin table)
                xq = [fp.tile([128, 1024], fp16, tag="xq", name=f"xq1{sfx}", bufs=2)]
                cq1 = fp.tile([128, 1024], fp16, tag="cq1", name=f"cq1{sfx}", bufs=2)
                for s in range(2):
                    nc.scalar.activation(
                        xq[0][:, s * 512 : (s + 1) * 512],
                        qp_ps[s][:],
                        Sin,
                        bias=hp[:, 0:1],
                        scale=1.0,
                    )
                    nc.scalar.activation(
                        cq1[:, s * 512 : (s + 1) * 512],
                        qp_ps[s][:],
                        Sin,
                        bias=hp[:, 1:2],
                        scale=1.0,
                    )
                xk = [fp.tile([128, 512], fp16, tag="xk", name=f"xk1{sfx}", bufs=2)]
                nc.scalar.activation(
                    xk[0][:], kp_ps[:], Sin, bias=hp[:, 2:3], scale=1.0
                )
                ck1 = fp.tile([128, 512], fp16, tag="ck1", name=f"ck1{sfx}", bufs=2)
                nc.scalar.activation(ck1[:], kp_ps[:], Sin, bias=hp[:, 1:2], scale=1.0)
                kk = fp.tile([128, 512], fp16, tag="kk", name=f"kk{sfx}", bufs=2)
                nc.scalar.activation(kk[:], kp_ps[:], Copy, scale=1.0 / OM1)

                # doubled-cos multiplier tiles
                dq = fp.tile([128, 1024], fp16, tag="dq", name=f"dq{sfx}", bufs=2)
                nc.vector.tensor_scalar_mul(dq[:], cq1[:], 2.0)
                dk = fp.tile([128, 512], fp16, tag="dk", name=f"dk{sfx}", bufs=2)
                nc.vector.tensor_scalar_mul(dk[:], ck1[:], 2.0)

                # ---- scaled k-features for r=1 (ACT, Sin-table Copy)
                bk = [fp.tile([128, 512], fp16, tag="bk1", name=f"bk1{sfx}", bufs=2)]
                nc.scalar.activation(bk[0][:], xk[0][:], Copy, scale=wbs[:, 0:1])

                # ---- beta matmuls (linear-term bias per key)
                beta_ps = psp.tile([128, 4], fp32, tag="beta", name=f"beta{sfx}")
                for j in range(NSLOT):
                    nc.tensor.matmul(
                        beta_ps[:, j : j + 1],
                        kk[:, j * 128 : (j + 1) * 128],
                        wvec[:],
                        start=True,
                        stop=True,
                    )
                betam = fp.tile([128, 4], fp32, tag="betam", name=f"betam{sfx}", bufs=2)
                for j in range(NSLOT):
                    nc.vector.tensor_tensor(
                        betam[:, j : j + 1],
                        beta_ps[:, j : j + 1],
                        mask_sb[:, j : j + 1],
                        add,
                    )

                # ---- ladders (DVE) + scaled copies (ACT) + S passes (PE)
                # PSUM bank budget is 8: reuse the projection banks for the
                # first three S accumulators (their consumers all run early).
                stag = ("qp0", "qp1", "kp", "S3")
                S_ps = [
                    psp.tile([128, 512], fp32, tag=stag[j], name=f"S{j}{sfx}")
                    for j in range(NSLOT)
                ]
                setof = (0, 0, 0, 1)
                for j in range(NSLOT):
                    s = setof[j]
                    nc.tensor.matmul(
                        S_ps[j][:],
                        bk[0][:, j * 128 : (j + 1) * 128],
                        xq[0][:, s * 512 : (s + 1) * 512],
                        start=True,
                        stop=False,
                    )
                for r in range(2, R + 1):
                    # q-side ladder step
                    tq = fp.tile(
                        [128, 1024], fp16, tag="tq", name=f"tq{r}{sfx}", bufs=2
                    )
                    nc.vector.tensor_tensor(tq[:], dq[:], xq[-1][:], mult)
                    xq.append(
                        fp.tile([128, 1024], fp16, tag=f"xq{r % 3}", name=f"xq{r}{sfx}", bufs=2)
                    )
                    prev2q = x0q if r == 2 else xq[-3]
                    nc.vector.tensor_tensor(xq[-1][:], tq[:], prev2q[:], sub)
                    # k-side ladder step (high harmonics on the idle Pool
                    # engine; DVE carries the rest)
                    keng = nc.gpsimd if r >= 10 else nc.vector
                    tk = fp.tile(
                        [128, 512], fp16, tag="tk", name=f"tk{r}{sfx}", bufs=2
                    )
                    keng.tensor_tensor(tk[:], dk[:], xk[-1][:], mult)
                    xk.append(
                        fp.tile([128, 512], fp16, tag=f"xk{r % 3}", name=f"xk{r}{sfx}", bufs=2)
                    )
                    prev2k = x0k if r == 2 else xk[-3]
                    keng.tensor_tensor(xk[-1][:], tk[:], prev2k[:], sub)
                    # scaled k-features
                    bk.append(
                        fp.tile(
                            [128, 512], fp16, tag=f"bk{r % 2}", name=f"bk{r}{sfx}", bufs=2
                        )
                    )
                    nc.scalar.activation(
                        bk[-1][:], xk[-1][:], Copy, scale=wbs[:, r - 1 : r]
                    )
                    # S passes for all four jobs at this harmonic
                    for j in range(NSLOT):
                        s = setof[j]
                        nc.tensor.matmul(
                            S_ps[j][:],
                            bk[-1][:, j * 128 : (j + 1) * 128],
                            xq[-1][:, s * 512 : (s + 1) * 512],
                            start=False,
                            stop=(r == R),
                        )

                # ---- softmax numerator + weighted values (l rides as row 64
                # of the A-half O matmul via the ones column in vtsA)
                oa_sb = fp.tile(
                    [65, NSLOT * 512], fp32, tag="oasb", name=f"oa{sfx}", bufs=1
                )
                ob_sb = fp.tile(
                    [64, NSLOT * 512], fp32, tag="obsb", name=f"ob{sfx}", bufs=1
                )
                for j in range(NSLOT):
                    P_j = fp.tile(
                        [128, 512], fp16, tag=f"P{j % 2}", name=f"P{j}{sfx}", bufs=2
                    )
                    nc.scalar.activation(
                        P_j[:], S_ps[j][:], Exp, bias=betam[:, j : j + 1], scale=1.0
                    )
                    Oa_ps = psp.tile([65, 512], fp32, tag="Oa", name=f"Oa{j}{sfx}")
                    nc.tensor.matmul(
                        Oa_ps[:],
                        vtsA[:, j * 65 : (j + 1) * 65],
                        P_j[:],
                        start=True,
                        stop=True,
                    )
                    Ob_ps = psp.tile([64, 512], fp32, tag="Ob", name=f"Ob{j}{sfx}")
                    nc.tensor.matmul(
                        Ob_ps[:],
                        vtsB[:, j * 64 : (j + 1) * 64],
                        P_j[:],
                        start=True,
                        stop=True,
                    )
                    nc.scalar.activation(
                        oa_sb[:, j * 512 : (j + 1) * 512], Oa_ps[:], Copy, scale=1.0
                    )
                    nc.vector.tensor_copy(ob_sb[:, j * 512 : (j + 1) * 512], Ob_ps[:])
                    if rep == repeat - 1:
                        nc.sync.dma_start(
                            outAE[:, j * 512 : (j + 1) * 512],
                            oa_sb[:, j * 512 : (j + 1) * 512],
                        )
                        nc.sync.dma_start(
                            outBE[:, j * 512 : (j + 1) * 512],
                            ob_sb[:, j * 512 : (j + 1) * 512],
                        )

    nc.compile()
    return nc


def _prepare(inputs):
    import ml_dtypes

    f16 = np.float16
    queries = np.asarray(inputs["queries"], dtype=np.float32)
    keys = np.asarray(inputs["keys"], dtype=np.float32)
    values = np.asarray(inputs["values"], dtype=np.float32)
    valid_lens = np.asarray(inputs["valid_lens"]).astype(np.int64)
    W_q = np.asarray(inputs["W_q"], dtype=np.float32)
    W_k = np.asarray(inputs["W_k"], dtype=np.float32)
    w_v = np.asarray(inputs["w_v"], dtype=np.float32)

    c1lin, bco = _fit_coeffs()
    cores = _plan([int(x) for x in valid_lens])

    wq2 = np.empty((128, 256), f16)
    wk2 = np.empty((128, 256), f16)
    for c in range(2):
        wq2[:, c * 128 : (c + 1) * 128] = np.tile(
            W_q[c * 128 : (c + 1) * 128] * OM1, (1, 2)
        ).astype(f16)
        wk2[:, c * 128 : (c + 1) * 128] = np.tile(
            W_k[c * 128 : (c + 1) * 128] * OM1, (1, 2)
        ).astype(f16)
    wvec = np.tile((0.5 * c1lin * w_v)[:, None], (2, 1)).astype(f16)
    wbs = np.tile(w_v[:, None] * bco[None, :], (2, 1)).astype(np.float32)
    hp = np.zeros((128, 3), np.float32)
    hp[64:128, 0] = np.pi / 2
    hp[:, 1] = np.pi / 2
    hp[0:64, 2] = np.pi / 2

    qT = {b: np.ascontiguousarray(queries[b].T) for b in range(B)}

    in_maps = []
    for c in range(NCORES):
        jobs = cores[c]
        qts = np.zeros((128, 2048), f16)
        for s, j in ((0, 0), (1, 3)):
            b, qh, _, _ = jobs[j]
            for ch in range(2):
                qts[:, s * 1024 + ch * 512 : s * 1024 + (ch + 1) * 512] = qT[b][
                    ch * 128 : (ch + 1) * 128, qh * 512 : (qh + 1) * 512
                ].astype(f16)
        kts = np.zeros((128, 1024), f16)
        vtsA = np.zeros((128, NSLOT * 65), f16)
        vtsB = np.zeros((128, NSLOT * 64), f16)
        mask = np.full((128, 4), MASK_VAL, np.float32)
        for j, (b, qh, s0, cnt) in enumerate(jobs):
            vtsA[:, j * 65 + 64] = 1.0  # ones column -> l row
            if cnt == 0:
                continue
            kp = np.zeros((128, D_IN), np.float32)
            kp[0:cnt] = keys[b, s0 : s0 + cnt]
            kT = kp.T  # (256, 128)
            for ch in range(2):
                kts[:, ch * 512 + j * 128 : ch * 512 + (j + 1) * 128] = kT[
                    ch * 128 : (ch + 1) * 128
                ].astype(f16)
            vp = np.zeros((128, D_V), np.float32)
            vp[0:cnt] = values[b, s0 : s0 + cnt]
            vtsA[:, j * 65 : j * 65 + 64] = vp[:, 0:64].astype(f16)
            vtsB[:, j * 64 : (j + 1) * 64] = vp[:, 64:128].astype(f16)
            mask[0:cnt, j] = 0.0
        in_maps.append(
            {
                "qts": qts,
                "kts": kts,
                "vtsA": vtsA,
                "vtsB": vtsB,
                "mask": mask,
                "wq2": wq2,
                "wk2": wk2,
                "wvec": wvec,
                "wbs": wbs,
                "hp": hp,
            }
        )
    return cores, in_maps


def kernel(**inputs):
    global LAST_RESULT
    cores, in_maps = _prepare(inputs)

    if "nc" not in _CACHE:
        _CACHE["nc"] = _build()
    nc = _CACHE["nc"]

    from concourse.bass_utils import run_bass_kernel_spmd

    res = run_bass_kernel_spmd(nc, in_maps, core_ids=list(range(NCORES)))
    LAST_RESULT = res

    O = np.zeros((B, D_V, Q), np.float64)
    L = np.zeros((B, Q), np.float64)
    for c in range(NCORES):
        oA = np.asarray(res.results[c]["outA"]).astype(np.float64)
        oB = np.asarray(res.results[c]["outB"]).astype(np.float64)
        for j, (b, qh, s0, cnt) in enumerate(cores[c]):
            if cnt == 0:
                continue
            sl = slice(j * 512, (j + 1) * 512)
            qs = slice(qh * 512, (qh + 1) * 512)
            O[b][0:64, qs] += oA[0:64, sl]
            O[b][64:128, qs] += oB[:, sl]
            L[b][qs] += oA[64, sl]
    out = (O / L[:, None, :]).transpose(0, 2, 1)
    return np.ascontiguousarray(out.astype(np.float32))


# revision 33
# speedup vs baseline: 1.0036x; 1.0036x over previous
"""Additive (Bahdanau) attention on 8 TRN2 NeuronCores — separable-score
formulation.

Math per batch b:
    q = queries[b] @ W_q                  (Q, H)
    k = keys[b]    @ W_k                  (K, H)
    S[i, j] = sum_h w_v[h] * tanh(q[i,h] + k[j,h])
    out[b]  = softmax_j(S masked) @ values[b]

Instead of materializing the (Q, K, H) tanh tensor (elementwise-engine
bound), tanh is approximated by a regularized Fourier-extension series

    tanh(u) ~= c1*u + sum_{r=1..R} b_r sin(r*w1*u),   w1 = pi/PERIOD

whose angle-addition expansion is separable in (q, k):

    S[i,j] ~= sum_h sum_r (w_h b_r) [sin(r w1 q) cos(r w1 k)
                                     + cos(r w1 q) sin(r w1 k)]
              + c1 * sum_h w_h k_jh   (+ a per-row term softmax discards)

so S becomes R chained 128-contraction matmuls over trig features, and the
linear term folds into the exp bias.  The HW Sin table is only valid for
|arg| <= pi, so just sin/cos(w1 q) are computed on ACT (|w1 q| + pi/2 < pi)
and higher harmonics come from the Chebyshev angle ladder on DVE in fp16:

    X_r = 2 cos(w1 x) * X_{r-1} - X_{r-2}

which runs the sin and cos chains together in one (128, n) tile
(partitions = 64 h x {sin, cos}).  Rel err of the whole approximation
(incl. fp16 features) ~= 5e-3 vs the exact reference.

Sharding: the prefix mask means only sum(valid_lens) key columns carry
work.  Valid keys are split into <=128-key jobs; each of the 8 cores runs
4 job slots fed by two query feature sets (set0 serves slots 0-2, set1
serves slot 3; a set is one (batch, q-half) of 512 queries).  Each job
emits unnormalized partials (O^T = V^T P, l = 1^T P); the host sums
partials per batch and divides.
"""

import sys

sys.path.insert(0, "/opt/trn_rl_repo")

import numpy as np

B, Q, KLEN, D_IN, H, D_V = 4, 1024, 1024, 256, 64, 128
NCORES = 8
NSLOT = 4  # job slots per core: slots 0-2 -> set0, slot 3 -> set1
MASK_VAL = -1.0e6

R = 11  # Fourier harmonics
PERIOD = 13.0
OM1 = np.pi / PERIOD
UFIT = 9.8
LAM = 0.1

_CACHE = {}
_COEF = None
LAST_RESULT = None


def _fit_coeffs():
    """Host-side (weights-only) fit: tanh(u) ~= c1*u + sum b_r sin(r w1 u)."""
    global _COEF
    if _COEF is not None:
        return _COEF
    u = np.linspace(-UFIT, UFIT, 6001)
    w = 0.25 + np.exp(-(u**2) / (2 * 3.0**2))
    cols = [u / UFIT] + [np.sin((r + 1) * np.pi * u / PERIOD) for r in range(R)]
    A = np.stack(cols, axis=1)
    AtA = (A * w[:, None] ** 2).T @ A + LAM * np.eye(A.shape[1])
    Aty = (A * w[:, None] ** 2).T @ np.tanh(u)
    coef = np.linalg.solve(AtA, Aty)
    _COEF = (coef[0] / UFIT, coef[1:].copy())
    return _COEF


def _plan(vl):
    """Split each batch's valid-key prefix into <=128-key jobs per q-half and
    pack them into 8 cores x (one 3-job set0 + one 1-job set1).

    Returns cores: list of 8 entries, each a list of NSLOT jobs
    (b, qh, start, cnt) with cnt == 0 for padding slots."""
    combos = []  # ((b, qh), [(start, cnt), ...])
    for b, v in enumerate(vl):
        nb = -(-v // 128) if v > 0 else 0
        chunks = []
        s = 0
        for i in range(nb):
            cnt = min(128, v - s)
            chunks.append((s, cnt))
            s += cnt
        for qh in range(2):
            if chunks:
                combos.append(((b, qh), list(chunks)))
    assert sum(len(c[1]) for c in combos) <= 4 * NCORES, (
        "job count exceeds slot capacity; vl sum too large for this plan"
    )
    # Greedy: fill 8 triple-slots (set0) and 8 single-slots (set1).
    triples = []  # (combo, [jobs up to 3])
    singles = []  # (combo, job)
    combos = sorted(combos, key=lambda c: -len(c[1]))
    leftovers = []
    for key, chunks in combos:
        i = 0
        while len(chunks) - i >= 3 and len(triples) < NCORES:
            triples.append((key, chunks[i : i + 3]))
            i += 3
        leftovers.append((key, chunks[i:]))
    # place leftovers: prefer singles; group of 2-3 can take a triple slot
    for key, chunks in sorted(leftovers, key=lambda c: -len(c[1])):
        i = 0
        while len(chunks) - i >= 2 and len(triples) < NCORES:
            take = min(3, len(chunks) - i)
            triples.append((key, chunks[i : i + take]))
            i += take
        while i < len(chunks):
            if len(singles) < NCORES:
                singles.append((key, chunks[i]))
            elif len(triples) < NCORES:
                triples.append((key, [chunks[i]]))
            else:
                raise AssertionError("packing failed")
            i += 1
    while len(triples) < NCORES:
        triples.append(((0, 0), []))
    while len(singles) < NCORES:
        singles.append(((0, 0), None))
    cores = []
    for c in range(NCORES):
        (b0, qh0), t_jobs = triples[c]
        (b1, qh1), s_job = singles[c]
        jobs = []
        for i in range(3):
            if i < len(t_jobs):
                jobs.append((b0, qh0, t_jobs[i][0], t_jobs[i][1]))
            else:
                jobs.append((b0, qh0, 0, 0))
        if s_job is not None:
            jobs.append((b1, qh1, s_job[0], s_job[1]))
        else:
            jobs.append((b1, qh1, 0, 0))
        cores.append(jobs)
    return cores


def _build(repeat=1):
    import concourse.tile as tile
    from concourse import bacc, mybir

    fp32 = mybir.dt.float32
    fp16 = mybir.dt.float16
    Sin = mybir.ActivationFunctionType.Sin
    Exp = mybir.ActivationFunctionType.Exp
    Copy = mybir.ActivationFunctionType.Copy
    mult = mybir.AluOpType.mult
    sub = mybir.AluOpType.subtract
    add = mybir.AluOpType.add

    nc = bacc.Bacc(
        "TRN2", target_bir_lowering=False, debug=False, num_devices=NCORES
    )
    qtsE = nc.dram_tensor("qts", [128, 2048], fp16, kind="ExternalInput").ap()
    ktsE = nc.dram_tensor("kts", [128, 1024], fp16, kind="ExternalInput").ap()
    # vtsA: per job 65 cols = [V[:, 0:64] | ones]; the ones column makes the
    # O-matmul emit l (= 1^T P) as output row 64.  vtsB: V[:, 64:128].
    vtsAE = nc.dram_tensor("vtsA", [128, NSLOT * 65], fp16, kind="ExternalInput").ap()
    vtsBE = nc.dram_tensor("vtsB", [128, NSLOT * 64], fp16, kind="ExternalInput").ap()
    maskE = nc.dram_tensor("mask", [128, 4], fp32, kind="ExternalInput").ap()
    wq2E = nc.dram_tensor("wq2", [128, 256], fp16, kind="ExternalInput").ap()
    wk2E = nc.dram_tensor("wk2", [128, 256], fp16, kind="ExternalInput").ap()
    wvecE = nc.dram_tensor("wvec", [128, 1], fp16, kind="ExternalInput").ap()
    wbsE = nc.dram_tensor("wbs", [128, R], fp32, kind="ExternalInput").ap()
    hpE = nc.dram_tensor("hp", [128, 3], fp32, kind="ExternalInput").ap()
    bf16 = mybir.dt.bfloat16
    outAE = nc.dram_tensor("outA", [65, NSLOT * 512], bf16, kind="ExternalOutput").ap()
    outBE = nc.dram_tensor("outB", [64, NSLOT * 512], bf16, kind="ExternalOutput").ap()

    with tile.TileContext(nc) as tc:
        with (
            tc.tile_pool(name="const", bufs=1) as cp,
            tc.tile_pool(name="feat", bufs=2) as fp,
            tc.tile_pool(name="ps", bufs=1, space="PSUM") as psp,
        ):
            # ---- input DMAs (q-path first), spread across rings
            qts = cp.tile([128, 2048], fp16)
            nc.sync.dma_start(qts[:, 0:512], qtsE[:, 0:512])
            wq2 = cp.tile([128, 256], fp16)
            nc.scalar.dma_start(wq2[:], wq2E[:, :])
            nc.sync.dma_start(qts[:, 512:1024], qtsE[:, 512:1024])
            nc.sync.dma_start(qts[:, 1024:1536], qtsE[:, 1024:1536])
            nc.sync.dma_start(qts[:, 1536:2048], qtsE[:, 1536:2048])
            kts = cp.tile([128, 1024], fp16)
            nc.scalar.dma_start(kts[:], ktsE[:, :])
            wk2 = cp.tile([128, 256], fp16)
            nc.scalar.dma_start(wk2[:], wk2E[:, :])
            hp = cp.tile([128, 3], fp32)
            nc.sync.dma_start(hp[:], hpE[:, :])
            wbs = cp.tile([128, R], fp32)
            nc.sync.dma_start(wbs[:], wbsE[:, :])
            vtsA = cp.tile([128, NSLOT * 65], fp16)
            nc.gpsimd.dma_start(vtsA[:], vtsAE[:, :])
            vtsB = cp.tile([128, NSLOT * 64], fp16)
            nc.gpsimd.dma_start(vtsB[:], vtsBE[:, :])
            mask_sb = cp.tile([128, 4], fp32)
            nc.sync.dma_start(mask_sb[:], maskE[:, :])
            wvec = cp.tile([128, 1], fp16)
            nc.scalar.dma_start(wvec[:], wvecE[:, :])

            # ladder r=0 tiles: q-side [sin0; cos0] = [0; 1], k-side [cos0; sin0]
            # (DVE memsets: DVE is idle until the anchors land anyway)
            x0q = cp.tile([128, 1024], fp16, name="x0q")
            nc.vector.memset(x0q[0:64, :], 0.0)
            nc.vector.memset(x0q[64:128, :], 1.0)
            x0k = cp.tile([128, 512], fp16, name="x0k")
            nc.vector.memset(x0k[0:64, :], 1.0)
            nc.vector.memset(x0k[64:128, :], 0.0)

            for rep in range(repeat):
                sfx = f"_{rep}"
                # ---- projections (stationaries duplicated so psum has both
                # halves: rows 0:64 and 64:128 hold identical h-values)
                qp_ps = [
                    psp.tile([128, 512], fp32, tag=f"qp{s}", name=f"qp{s}{sfx}")
                    for s in range(2)
                ]
                for s in range(2):
                    for c in range(2):
                        nc.tensor.matmul(
                            qp_ps[s][:],
                            wq2[:, c * 128 : (c + 1) * 128],
                            qts[:, s * 1024 + c * 512 : s * 1024 + (c + 1) * 512],
                            start=(c == 0),
                            stop=(c == 1),
                        )
                kp_ps = psp.tile([128, 512], fp32, tag="kp", name=f"kp{sfx}")
                for c in range(2):
                    nc.tensor.matmul(
                        kp_ps[:],
                        wk2[:, c * 128 : (c + 1) * 128],
                        kts[:, c * 512 : (c + 1) * 512],
                        start=(c == 0),
                        stop=(c == 1),
                    )

                # ---- ACT anchors (Sin table)
                xq = [fp.tile([128, 1024], fp16, tag="xq", name=f"xq1{sfx}", bufs=2)]
                cq1 = fp.tile([128, 1024], fp16, tag="cq1", name=f"cq1{sfx}", bufs=2)
                for s in range(2):
                    nc.scalar.activation(
                        xq[0][:, s * 512 : (s + 1) * 512],
                        qp_ps[s][:],
                        Sin,
                        bias=hp[:, 0:1],
                        scale=1.0,
                    )
                    nc.scalar.activation(
                        cq1[:, s * 512 : (s + 1) * 512],
                        qp_ps[s][:],
                        Sin,
                        bias=hp[:, 1:2],
                        scale=1.0,
                    )
                xk = [fp.tile([128, 512], fp16, tag="xk", name=f"xk1{sfx}", bufs=2)]
                nc.scalar.activation(
                    xk[0][:], kp_ps[:], Sin, bias=hp[:, 2:3], scale=1.0
                )
                ck1 = fp.tile([128, 512], fp16, tag="ck1", name=f"ck1{sfx}", bufs=2)
                nc.scalar.activation(ck1[:], kp_ps[:], Sin, bias=hp[:, 1:2], scale=1.0)
                kk = fp.tile([128, 512], fp16, tag="kk", name=f"kk{sfx}", bufs=2)
                nc.scalar.activation(kk[:], kp_ps[:], Copy, scale=1.0 / OM1)

                # doubled-cos multiplier tiles
                dq = fp.tile([128, 1024], fp16, tag="dq", name=f"dq{sfx}", bufs=2)
                nc.vector.tensor_scalar_mul(dq[:], cq1[:], 2.0)
                dk = fp.tile([128, 512], fp16, tag="dk", name=f"dk{sfx}", bufs=2)
                nc.vector.tensor_scalar_mul(dk[:], ck1[:], 2.0)

                # ---- scaled k-features for r=1 (ACT, Sin-table Copy)
                bk = [fp.tile([128, 512], fp16, tag="bk1", name=f"bk1{sfx}", bufs=2)]
                nc.scalar.activation(bk[0][:], xk[0][:], Copy, scale=wbs[:, 0:1])

                # ---- beta matmuls (linear-term bias per key)
                beta_ps = psp.tile([128, 4], fp32, tag="beta", name=f"beta{sfx}")
                for j in range(NSLOT):
                    nc.tensor.matmul(
                        beta_ps[:, j : j + 1],
                        kk[:, j * 128 : (j + 1) * 128],
                        wvec[:],
                        start=True,
                        stop=True,
                    )
                betam = fp.tile([128, 4], fp32, tag="betam", name=f"betam{sfx}", bufs=2)
                for j in range(NSLOT):
                    nc.vector.tensor_tensor(
                        betam[:, j : j + 1],
                        beta_ps[:, j : j + 1],
                        mask_sb[:, j : j + 1],
                        add,
                    )

                # ---- ladders (DVE) + scaled copies (ACT) + S passes (PE)
                # PSUM bank budget is 8: qp0, qp1, kp, beta + S0-3.  The Oa/Ob
                # output accumulators cycle through the S banks once each S is
                # consumed by its exp, so next rep's projections never wait on
                # this rep's tail.
                S_ps = [
                    psp.tile([128, 512], fp32, tag=f"S{j}", name=f"S{j}{sfx}")
                    for j in range(NSLOT)
                ]
                setof = (0, 0, 0, 1)
                for j in range(NSLOT):
                    s = setof[j]
                    nc.tensor.matmul(
                        S_ps[j][:],
                        bk[0][:, j * 128 : (j + 1) * 128],
                        xq[0][:, s * 512 : (s + 1) * 512],
                        start=True,
                        stop=False,
                    )
                # k-side ladder first (high priority so the scheduler finishes
                # the k-chain early: the last bk copy gates the Exp table
                # load, which otherwise lands on the critical tail).
                KSPLIT = R
                with tc.high_priority():
                    for r in range(2, R + 1):
                        keng = nc.vector if r <= KSPLIT else nc.gpsimd
                        tk = fp.tile(
                            [128, 512], fp16, tag="tk", name=f"tk{r}{sfx}", bufs=2
                        )
                        keng.tensor_tensor(tk[:], dk[:], xk[-1][:], mult)
                        xk.append(
                            fp.tile(
                                [128, 512], fp16, tag=f"xk{r}", name=f"xk{r}{sfx}", bufs=2
                            )
                        )
                        prev2k = x0k if r == 2 else xk[-3]
                        keng.tensor_tensor(xk[-1][:], tk[:], prev2k[:], sub)
                        # scaled k-features
                        bk.append(
                            fp.tile(
                                [128, 512], fp16, tag=f"bk{r}", name=f"bk{r}{sfx}", bufs=2
                            )
                        )
                        nc.scalar.activation(
                            bk[-1][:], xk[-1][:], Copy, scale=wbs[:, r - 1 : r]
                        )
                QSPL = 768  # q-ladder col split: [0:QSPL] DVE, [QSPL:] Pool
                for r in range(2, R + 1):
                    # q-side ladder step, column-split across DVE and Pool
                    tq = fp.tile(
                        [128, 1024], fp16, tag="tq", name=f"tq{r}{sfx}", bufs=2
                    )
                    nc.vector.tensor_tensor(
                        tq[:, 0:QSPL], dq[:, 0:QSPL], xq[-1][:, 0:QSPL], mult
                    )
                    nc.gpsimd.tensor_tensor(
                        tq[:, QSPL:1024], dq[:, QSPL:1024], xq[-1][:, QSPL:1024], mult
                    )
                    xq.append(
                        fp.tile([128, 1024], fp16, tag=f"xq{r % 3}", name=f"xq{r}{sfx}", bufs=2)
                    )
                    prev2q = x0q if r == 2 else xq[-3]
                    nc.vector.tensor_tensor(
                        xq[-1][:, 0:QSPL], tq[:, 0:QSPL], prev2q[:, 0:QSPL], sub
                    )
                    nc.gpsimd.tensor_tensor(
                        xq[-1][:, QSPL:1024], tq[:, QSPL:1024], prev2q[:, QSPL:1024], sub
                    )
                    # S passes for all four jobs at this harmonic
                    for j in range(NSLOT):
                        s = setof[j]
                        nc.tensor.matmul(
                            S_ps[j][:],
                            bk[r - 1][:, j * 128 : (j + 1) * 128],
                            xq[-1][:, s * 512 : (s + 1) * 512],
                            start=False,
                            stop=(r == R),
                        )

                # ---- softmax numerator + weighted values (l rides as row 64
                # of the A-half O matmul via the ones column in vtsA)
                oa_sb = fp.tile(
                    [65, NSLOT * 512], bf16, tag="oasb", name=f"oa{sfx}", bufs=1
                )
                ob_sb = fp.tile(
                    [64, NSLOT * 512], bf16, tag="obsb", name=f"ob{sfx}", bufs=1
                )
                for j in range(NSLOT):
                    P_j = fp.tile(
                        [128, 512], fp16, tag=f"P{j % 2}", name=f"P{j}{sfx}", bufs=2
                    )
                    nc.scalar.activation(
                        P_j[:], S_ps[j][:], Exp, bias=betam[:, j : j + 1], scale=1.0
                    )
                    Oa_ps = psp.tile(
                        [128, 512], fp32, tag=f"S{j}", name=f"Oa{j}{sfx}"
                    )
                    nc.tensor.matmul(
                        Oa_ps[0:65, :],
                        vtsA[:, j * 65 : (j + 1) * 65],
                        P_j[:],
                        start=True,
                        stop=True,
                    )
                    Ob_ps = psp.tile(
                        [128, 512], fp32, tag=f"S{(j + 1) % NSLOT}", name=f"Ob{j}{sfx}"
                    )
                    nc.tensor.matmul(
                        Ob_ps[0:64, :],
                        vtsB[:, j * 64 : (j + 1) * 64],
                        P_j[:],
                        start=True,
                        stop=True,
                    )
                    nc.vector.tensor_scalar_mul(
                        oa_sb[:, j * 512 : (j + 1) * 512], Oa_ps[0:65, :], 1.0
                    )
                    nc.scalar.activation(
                        ob_sb[:, j * 512 : (j + 1) * 512], Ob_ps[0:64, :], Copy,
                        scale=1.0,
                    )
                    if rep == repeat - 1:
                        ringA = nc.sync if j % 2 == 0 else nc.scalar
                        ringB = nc.scalar if j % 2 == 0 else nc.sync
                        ringA.dma_start(
                            outAE[:, j * 512 : (j + 1) * 512],
                            oa_sb[:, j * 512 : (j + 1) * 512],
                        )
                        ringB.dma_start(
                            outBE[:, j * 512 : (j + 1) * 512],
                            ob_sb[:, j * 512 : (j + 1) * 512],
                        )

    nc.compile()
    return nc


def _prepare(inputs):
    import ml_dtypes

    f16 = np.float16
    queries = np.asarray(inputs["queries"], dtype=np.float32)
    keys = np.asarray(inputs["keys"], dtype=np.float32)
    values = np.asarray(inputs["values"], dtype=np.float32)
    valid_lens = np.asarray(inputs["valid_lens"]).astype(np.int64)
    W_q = np.asarray(inputs["W_q"], dtype=np.float32)
    W_k = np.asarray(inputs["W_k"], dtype=np.float32)
    w_v = np.asarray(inputs["w_v"], dtype=np.float32)

    c1lin, bco = _fit_coeffs()
    cores = _plan([int(x) for x in valid_lens])

    wq2 = np.empty((128, 256), f16)
    wk2 = np.empty((128, 256), f16)
    for c in range(2):
        wq2[:, c * 128 : (c + 1) * 128] = np.tile(
            W_q[c * 128 : (c + 1) * 128] * OM1, (1, 2)
        ).astype(f16)
        wk2[:, c * 128 : (c + 1) * 128] = np.tile(
            W_k[c * 128 : (c + 1) * 128] * OM1, (1, 2)
        ).astype(f16)
    wvec = np.tile((0.5 * c1lin * w_v)[:, None], (2, 1)).astype(f16)
    wbs = np.tile(w_v[:, None] * bco[None, :], (2, 1)).astype(np.float32)
    hp = np.zeros((128, 3), np.float32)
    hp[64:128, 0] = np.pi / 2
    hp[:, 1] = np.pi / 2
    hp[0:64, 2] = np.pi / 2

    qT = {b: np.ascontiguousarray(queries[b].T) for b in range(B)}

    in_maps = []
    for c in range(NCORES):
        jobs = cores[c]
        qts = np.zeros((128, 2048), f16)
        for s, j in ((0, 0), (1, 3)):
            b, qh, _, _ = jobs[j]
            for ch in range(2):
                qts[:, s * 1024 + ch * 512 : s * 1024 + (ch + 1) * 512] = qT[b][
                    ch * 128 : (ch + 1) * 128, qh * 512 : (qh + 1) * 512
                ].astype(f16)
        kts = np.zeros((128, 1024), f16)
        vtsA = np.zeros((128, NSLOT * 65), f16)
        vtsB = np.zeros((128, NSLOT * 64), f16)
        mask = np.full((128, 4), MASK_VAL, np.float32)
        for j, (b, qh, s0, cnt) in enumerate(jobs):
            vtsA[:, j * 65 + 64] = 1.0  # ones column -> l row
            if cnt == 0:
                continue
            kp = np.zeros((128, D_IN), np.float32)
            kp[0:cnt] = keys[b, s0 : s0 + cnt]
            kT = kp.T  # (256, 128)
            for ch in range(2):
                kts[:, ch * 512 + j * 128 : ch * 512 + (j + 1) * 128] = kT[
                    ch * 128 : (ch + 1) * 128
                ].astype(f16)
            vp = np.zeros((128, D_V), np.float32)
            vp[0:cnt] = values[b, s0 : s0 + cnt]
            vtsA[:, j * 65 : j * 65 + 64] = vp[:, 0:64].astype(f16)
            vtsB[:, j * 64 : (j + 1) * 64] = vp[:, 64:128].astype(f16)
            mask[0:cnt, j] = 0.0
        in_maps.append(
            {
                "qts": qts,
                "kts": kts,
                "vtsA": vtsA,
                "vtsB": vtsB,
                "mask": mask,
                "wq2": wq2,
                "wk2": wk2,
                "wvec": wvec,
                "wbs": wbs,
                "hp": hp,
            }
        )
    return cores, in_maps


def kernel(**inputs):
    global LAST_RESULT
    cores, in_maps = _prepare(inputs)

    if "nc" not in _CACHE:
        _CACHE["nc"] = _build()
    nc = _CACHE["nc"]

    from concourse.bass_utils import run_bass_kernel_spmd

    res = run_bass_kernel_spmd(nc, in_maps, core_ids=list(range(NCORES)))
    LAST_RESULT = res

    O = np.zeros((B, D_V, Q), np.float64)
    L = np.zeros((B, Q), np.float64)
    for c in range(NCORES):
        oA = np.asarray(res.results[c]["outA"]).astype(np.float64)
        oB = np.asarray(res.results[c]["outB"]).astype(np.float64)
        for j, (b, qh, s0, cnt) in enumerate(cores[c]):
            if cnt == 0:
                continue
            sl = slice(j * 512, (j + 1) * 512)
            qs = slice(qh * 512, (qh + 1) * 512)
            O[b][0:64, qs] += oA[0:64, sl]
            O[b][64:128, qs] += oB[:, sl]
            L[b][qs] += oA[64, sl]
    out = (O / L[:, None, :]).transpose(0, 2, 1)
    return np.ascontiguousarray(out.astype(np.float32))


# revision 45
# speedup vs baseline: 1.0182x; 1.0145x over previous
"""Additive (Bahdanau) attention on 8 TRN2 NeuronCores — separable-score
formulation.

Math per batch b:
    q = queries[b] @ W_q                  (Q, H)
    k = keys[b]    @ W_k                  (K, H)
    S[i, j] = sum_h w_v[h] * tanh(q[i,h] + k[j,h])
    out[b]  = softmax_j(S masked) @ values[b]

Instead of materializing the (Q, K, H) tanh tensor (elementwise-engine
bound), tanh is approximated by a regularized Fourier-extension series

    tanh(u) ~= c1*u + sum_{r=1..R} b_r sin(r*w1*u),   w1 = pi/PERIOD

whose angle-addition expansion is separable in (q, k):

    S[i,j] ~= sum_h sum_r (w_h b_r) [sin(r w1 q) cos(r w1 k)
                                     + cos(r w1 q) sin(r w1 k)]
              + c1 * sum_h w_h k_jh   (+ a per-row term softmax discards)

so S becomes R chained 128-contraction matmuls over trig features, and the
linear term folds into the exp bias.  The HW Sin table is only valid for
|arg| <= pi, so just sin/cos(w1 q) are computed on ACT (|w1 q| + pi/2 < pi)
and higher harmonics come from the Chebyshev angle ladder on DVE in fp16:

    X_r = 2 cos(w1 x) * X_{r-1} - X_{r-2}

which runs the sin and cos chains together in one (128, n) tile
(partitions = 64 h x {sin, cos}).  Rel err of the whole approximation
(incl. fp16 features) ~= 5e-3 vs the exact reference.

Sharding: the prefix mask means only sum(valid_lens) key columns carry
work.  Valid keys are split into <=128-key jobs; each of the 8 cores runs
4 job slots fed by two query feature sets (set0 serves slots 0-2, set1
serves slot 3; a set is one (batch, q-half) of 512 queries).  Each job
emits unnormalized partials (O^T = V^T P, l = 1^T P); the host sums
partials per batch and divides.
"""

import sys

sys.path.insert(0, "/opt/trn_rl_repo")

import numpy as np

B, Q, KLEN, D_IN, H, D_V = 4, 1024, 1024, 256, 64, 128
NCORES = 8
NSLOT = 4  # job slots per core: slots 0-2 -> set0, slot 3 -> set1
MASK_VAL = -1.0e6

R = 11  # Fourier harmonics
PERIOD = 13.0
OM1 = np.pi / PERIOD
UFIT = 9.8
LAM = 0.1

_CACHE = {}
_COEF = None
LAST_RESULT = None


def _fit_coeffs():
    """Host-side (weights-only) fit: tanh(u) ~= c1*u + sum b_r sin(r w1 u)."""
    global _COEF
    if _COEF is not None:
        return _COEF
    u = np.linspace(-UFIT, UFIT, 6001)
    w = 0.25 + np.exp(-(u**2) / (2 * 3.0**2))
    cols = [u / UFIT] + [np.sin((r + 1) * np.pi * u / PERIOD) for r in range(R)]
    A = np.stack(cols, axis=1)
    AtA = (A * w[:, None] ** 2).T @ A + LAM * np.eye(A.shape[1])
    Aty = (A * w[:, None] ** 2).T @ np.tanh(u)
    coef = np.linalg.solve(AtA, Aty)
    _COEF = (coef[0] / UFIT, coef[1:].copy())
    return _COEF


def _plan(vl):
    """Split each batch's valid-key prefix into <=128-key jobs per q-half and
    pack them into 8 cores x (one 3-job set0 + one 1-job set1).

    Returns cores: list of 8 entries, each a list of NSLOT jobs
    (b, qh, start, cnt) with cnt == 0 for padding slots."""
    combos = []  # ((b, qh), [(start, cnt), ...])
    for b, v in enumerate(vl):
        nb = -(-v // 128) if v > 0 else 0
        chunks = []
        s = 0
        for i in range(nb):
            cnt = min(128, v - s)
            chunks.append((s, cnt))
            s += cnt
        for qh in range(2):
            if chunks:
                combos.append(((b, qh), list(chunks)))
    assert sum(len(c[1]) for c in combos) <= 4 * NCORES, (
        "job count exceeds slot capacity; vl sum too large for this plan"
    )
    # Greedy: fill 8 triple-slots (set0) and 8 single-slots (set1).
    triples = []  # (combo, [jobs up to 3])
    singles = []  # (combo, job)
    combos = sorted(combos, key=lambda c: -len(c[1]))
    leftovers = []
    for key, chunks in combos:
        i = 0
        while len(chunks) - i >= 3 and len(triples) < NCORES:
            triples.append((key, chunks[i : i + 3]))
            i += 3
        leftovers.append((key, chunks[i:]))
    # place leftovers: prefer singles; group of 2-3 can take a triple slot
    for key, chunks in sorted(leftovers, key=lambda c: -len(c[1])):
        i = 0
        while len(chunks) - i >= 2 and len(triples) < NCORES:
            take = min(3, len(chunks) - i)
            triples.append((key, chunks[i : i + take]))
            i += take
        while i < len(chunks):
            if len(singles) < NCORES:
                singles.append((key, chunks[i]))
            elif len(triples) < NCORES:
                triples.append((key, [chunks[i]]))
            else:
                raise AssertionError("packing failed")
            i += 1
    while len(triples) < NCORES:
        triples.append(((0, 0), []))
    while len(singles) < NCORES:
        singles.append(((0, 0), None))
    cores = []
    for c in range(NCORES):
        (b0, qh0), t_jobs = triples[c]
        (b1, qh1), s_job = singles[c]
        jobs = []
        for i in range(3):
            if i < len(t_jobs):
                jobs.append((b0, qh0, t_jobs[i][0], t_jobs[i][1]))
            else:
                jobs.append((b0, qh0, 0, 0))
        if s_job is not None:
            jobs.append((b1, qh1, s_job[0], s_job[1]))
        else:
            jobs.append((b1, qh1, 0, 0))
        cores.append(jobs)
    return cores


def _build(repeat=1):
    import concourse.tile as tile
    from concourse import bacc, mybir

    fp32 = mybir.dt.float32
    fp16 = mybir.dt.float16
    Sin = mybir.ActivationFunctionType.Sin
    Exp = mybir.ActivationFunctionType.Exp
    Copy = mybir.ActivationFunctionType.Copy
    mult = mybir.AluOpType.mult
    sub = mybir.AluOpType.subtract
    add = mybir.AluOpType.add

    nc = bacc.Bacc(
        "TRN2", target_bir_lowering=False, debug=False, num_devices=NCORES
    )
    qtsE = nc.dram_tensor("qts", [128, 2048], fp16, kind="ExternalInput").ap()
    ktsE = nc.dram_tensor("kts", [128, 1024], fp16, kind="ExternalInput").ap()
    # vtsA: per job 65 cols = [V[:, 0:64] | ones]; the ones column makes the
    # O-matmul emit l (= 1^T P) as output row 64.  vtsB: V[:, 64:128].
    vtsAE = nc.dram_tensor("vtsA", [128, NSLOT * 65], fp16, kind="ExternalInput").ap()
    vtsBE = nc.dram_tensor("vtsB", [128, NSLOT * 64], fp16, kind="ExternalInput").ap()
    maskE = nc.dram_tensor("mask", [128, 4], fp32, kind="ExternalInput").ap()
    wq2E = nc.dram_tensor("wq2", [128, 256], fp16, kind="ExternalInput").ap()
    wk2E = nc.dram_tensor("wk2", [128, 256], fp16, kind="ExternalInput").ap()
    wvecE = nc.dram_tensor("wvec", [128, 1], fp16, kind="ExternalInput").ap()
    wbsE = nc.dram_tensor("wbs", [128, R], fp32, kind="ExternalInput").ap()
    hpE = nc.dram_tensor("hp", [128, 3], fp32, kind="ExternalInput").ap()
    bf16 = mybir.dt.bfloat16
    outAE = nc.dram_tensor("outA", [65, NSLOT * 512], bf16, kind="ExternalOutput").ap()
    outBE = nc.dram_tensor("outB", [64, NSLOT * 512], bf16, kind="ExternalOutput").ap()

    with tile.TileContext(nc) as tc:
        with (
            tc.tile_pool(name="const", bufs=1) as cp,
            tc.tile_pool(name="feat", bufs=2) as fp,
            tc.tile_pool(name="ps", bufs=1, space="PSUM") as psp,
        ):
            # ---- input DMAs (q-path first), spread across rings
            hp = cp.tile([128, 3], fp32)
            nc.gpsimd.dma_start(hp[:], hpE[:, :])
            wbs = cp.tile([128, R], fp32)
            nc.gpsimd.dma_start(wbs[:], wbsE[:, :])
            wk2 = cp.tile([128, 256], fp16)
            nc.scalar.dma_start(wk2[:], wk2E[:, :])
            kts = cp.tile([128, 1024], fp16)
            nc.scalar.dma_start(kts[:, 0:512], ktsE[:, 0:512])
            nc.sync.dma_start(kts[:, 512:1024], ktsE[:, 512:1024])
            wq2 = cp.tile([128, 256], fp16)
            nc.sync.dma_start(wq2[:], wq2E[:, :])
            qts = cp.tile([128, 2048], fp16)
            nc.sync.dma_start(qts[:, 0:512], qtsE[:, 0:512])
            nc.sync.dma_start(qts[:, 512:1024], qtsE[:, 512:1024])
            nc.scalar.dma_start(qts[:, 1024:1536], qtsE[:, 1024:1536])
            nc.scalar.dma_start(qts[:, 1536:2048], qtsE[:, 1536:2048])
            mask_sb = cp.tile([128, 4], fp32)
            nc.gpsimd.dma_start(mask_sb[:], maskE[:, :])
            wvec = cp.tile([128, 1], fp16)
            nc.gpsimd.dma_start(wvec[:], wvecE[:, :])
            vtsA = cp.tile([128, NSLOT * 65], fp16)
            nc.gpsimd.dma_start(vtsA[:], vtsAE[:, :])
            vtsB = cp.tile([128, NSLOT * 64], fp16)
            nc.gpsimd.dma_start(vtsB[:], vtsBE[:, :])

            # dummy 1-element activations: force the ACT table loads during
            # the input DMAs instead of on the anchor critical path
            with tc.high_priority():
                scrap = cp.tile([1, 1], fp16, name="scrap")
                nc.scalar.activation(scrap[:], hp[0:1, 0:1], Sin)
                nc.scalar.activation(
                    scrap[:], hp[0:1, 0:1], Copy, scale=wbs[0:1, 0:1]
                )
                nc.scalar.activation(
                    scrap[:], hp[0:1, 0:1], Sin, bias=hp[0:1, 1:2], scale=1.0
                )

            # ladder r=0 tiles: q-side [sin0; cos0] = [0; 1], k-side [cos0; sin0]
            # (DVE memsets: DVE is idle until the anchors land anyway)
            x0q = cp.tile([128, 1024], fp16, name="x0q")
            nc.vector.memset(x0q[0:64, :], 0.0)
            nc.vector.memset(x0q[64:128, :], 1.0)
            x0k = cp.tile([128, 512], fp16, name="x0k")
            nc.vector.memset(x0k[0:64, :], 1.0)
            nc.vector.memset(x0k[64:128, :], 0.0)

            for rep in range(repeat):
                sfx = f"_{rep}"
                # ---- projections (stationaries duplicated so psum has both
                # halves: rows 0:64 and 64:128 hold identical h-values)
                kp_ps = psp.tile([128, 512], fp32, tag="kp", name=f"kp{sfx}")
                for c in range(2):
                    nc.tensor.matmul(
                        kp_ps[:],
                        wk2[:, c * 128 : (c + 1) * 128],
                        kts[:, c * 512 : (c + 1) * 512],
                        start=(c == 0),
                        stop=(c == 1),
                    )
                qp_ps = [
                    psp.tile([128, 512], fp32, tag=f"qp{s}", name=f"qp{s}{sfx}")
                    for s in range(2)
                ]
                for s in range(2):
                    for c in range(2):
                        nc.tensor.matmul(
                            qp_ps[s][:],
                            wq2[:, c * 128 : (c + 1) * 128],
                            qts[:, s * 1024 + c * 512 : s * 1024 + (c + 1) * 512],
                            start=(c == 0),
                            stop=(c == 1),
                        )

                # ---- ACT anchors (Sin table), k-side first: the k-chain
                # gates the bk copies and thus the Exp table load.
                xk = [fp.tile([128, 512], fp16, tag="xk", name=f"xk1{sfx}", bufs=2)]
                nc.scalar.activation(
                    xk[0][:], kp_ps[:], Sin, bias=hp[:, 2:3], scale=1.0
                )
                ck1 = fp.tile([128, 512], fp16, tag="ck1", name=f"ck1{sfx}", bufs=2)
                nc.scalar.activation(ck1[:], kp_ps[:], Sin, bias=hp[:, 1:2], scale=1.0)
                # kk = scaled keys for the beta matmul (the 1/OM1 unscaling is
                # folded into wvec host-side)
                kk = fp.tile([128, 512], fp16, tag="kk", name=f"kk{sfx}", bufs=2)
                nc.scalar.activation(kk[:], kp_ps[:], Copy, scale=1.0)
                dk = fp.tile([128, 512], fp16, tag="dk", name=f"dk{sfx}", bufs=2)
                nc.vector.tensor_scalar_mul(dk[:], ck1[:], 2.0)

                xq = [fp.tile([128, 1024], fp16, tag="xq", name=f"xq1{sfx}", bufs=2)]
                cq1 = fp.tile([128, 1024], fp16, tag="cq1", name=f"cq1{sfx}", bufs=2)
                for s in range(2):
                    nc.scalar.activation(
                        xq[0][:, s * 512 : (s + 1) * 512],
                        qp_ps[s][:],
                        Sin,
                        bias=hp[:, 0:1],
                        scale=1.0,
                    )
                    nc.scalar.activation(
                        cq1[:, s * 512 : (s + 1) * 512],
                        qp_ps[s][:],
                        Sin,
                        bias=hp[:, 1:2],
                        scale=1.0,
                    )

                # doubled-cos multiplier tile (q side)
                dq = fp.tile([128, 1024], fp16, tag="dq", name=f"dq{sfx}", bufs=2)
                nc.vector.tensor_scalar_mul(dq[:], cq1[:], 2.0)

                # ---- scaled k-features for r=1 (ACT, Sin-table Copy)
                bk = [fp.tile([128, 512], fp16, tag="bk1", name=f"bk1{sfx}", bufs=2)]
                nc.scalar.activation(bk[0][:], xk[0][:], Copy, scale=wbs[:, 0:1])

                # ---- beta matmuls (linear-term bias per key)
                beta_ps = psp.tile([128, 4], fp32, tag="beta", name=f"beta{sfx}")
                for j in range(NSLOT):
                    nc.tensor.matmul(
                        beta_ps[:, j : j + 1],
                        kk[:, j * 128 : (j + 1) * 128],
                        wvec[:],
                        start=True,
                        stop=True,
                    )
                betam = fp.tile([128, 4], fp32, tag="betam", name=f"betam{sfx}", bufs=2)
                for j in range(NSLOT):
                    nc.vector.tensor_tensor(
                        betam[:, j : j + 1],
                        beta_ps[:, j : j + 1],
                        mask_sb[:, j : j + 1],
                        add,
                    )

                # ---- ladders (DVE) + scaled copies (ACT) + S passes (PE)
                # PSUM bank budget is 8: qp0, qp1, kp, beta + S0-3.  The Oa/Ob
                # output accumulators cycle through the S banks once each S is
                # consumed by its exp, so next rep's projections never wait on
                # this rep's tail.
                S_ps = [
                    psp.tile([128, 512], fp32, tag=f"S{j}", name=f"S{j}{sfx}")
                    for j in range(NSLOT)
                ]
                setof = (0, 0, 0, 1)
                for j in range(NSLOT):
                    s = setof[j]
                    nc.tensor.matmul(
                        S_ps[j][:],
                        bk[0][:, j * 128 : (j + 1) * 128],
                        xq[0][:, s * 512 : (s + 1) * 512],
                        start=True,
                        stop=False,
                    )
                # k-side ladder first (high priority so the scheduler finishes
                # the k-chain early: the last bk copy gates the Exp table
                # load, which otherwise lands on the critical tail).
                KSPLIT = R
                with tc.high_priority():
                    for r in range(2, R + 1):
                        keng = nc.vector if r <= KSPLIT else nc.gpsimd
                        tk = fp.tile(
                            [128, 512], fp16, tag="tk", name=f"tk{r}{sfx}", bufs=2
                        )
                        keng.tensor_tensor(tk[:], dk[:], xk[-1][:], mult)
                        xk.append(
                            fp.tile(
                                [128, 512], fp16, tag=f"xk{r}", name=f"xk{r}{sfx}", bufs=2
                            )
                        )
                        prev2k = x0k if r == 2 else xk[-3]
                        keng.tensor_tensor(xk[-1][:], tk[:], prev2k[:], sub)
                        # scaled k-features
                        bk.append(
                            fp.tile(
                                [128, 512], fp16, tag=f"bk{r}", name=f"bk{r}{sfx}", bufs=2
                            )
                        )
                        nc.scalar.activation(
                            bk[-1][:], xk[-1][:], Copy, scale=wbs[:, r - 1 : r]
                        )
                QSPL = 768  # q-ladder col split: [0:QSPL] DVE, [QSPL:] Pool
                for r in range(2, R + 1):
                    # q-side ladder step, column-split across DVE and Pool
                    tq = fp.tile(
                        [128, 1024], fp16, tag="tq", name=f"tq{r}{sfx}", bufs=2
                    )
                    nc.vector.tensor_tensor(
                        tq[:, 0:QSPL], dq[:, 0:QSPL], xq[-1][:, 0:QSPL], mult
                    )
                    nc.gpsimd.tensor_tensor(
                        tq[:, QSPL:1024], dq[:, QSPL:1024], xq[-1][:, QSPL:1024], mult
                    )
                    xq.append(
                        fp.tile([128, 1024], fp16, tag=f"xq{r % 3}", name=f"xq{r}{sfx}", bufs=2)
                    )
                    prev2q = x0q if r == 2 else xq[-3]
                    nc.vector.tensor_tensor(
                        xq[-1][:, 0:QSPL], tq[:, 0:QSPL], prev2q[:, 0:QSPL], sub
                    )
                    nc.gpsimd.tensor_tensor(
                        xq[-1][:, QSPL:1024], tq[:, QSPL:1024], prev2q[:, QSPL:1024], sub
                    )
                    # S passes for all four jobs at this harmonic
                    for j in range(NSLOT):
                        s = setof[j]
                        nc.tensor.matmul(
                            S_ps[j][:],
                            bk[r - 1][:, j * 128 : (j + 1) * 128],
                            xq[-1][:, s * 512 : (s + 1) * 512],
                            start=False,
                            stop=(r == R),
                        )

                # ---- softmax numerator + weighted values (l rides as row 64
                # of the A-half O matmul via the ones column in vtsA)
                oa_sb = fp.tile(
                    [65, NSLOT * 512], bf16, tag="oasb", name=f"oa{sfx}", bufs=1
                )
                ob_sb = fp.tile(
                    [64, NSLOT * 512], bf16, tag="obsb", name=f"ob{sfx}", bufs=1
                )
                for j in range(NSLOT):
                    P_j = fp.tile(
                        [128, 512], fp16, tag=f"P{j % 2}", name=f"P{j}{sfx}", bufs=2
                    )
                    nc.scalar.activation(
                        P_j[:], S_ps[j][:], Exp, bias=betam[:, j : j + 1], scale=1.0
                    )
                    Oa_ps = psp.tile(
                        [128, 512], fp32, tag=f"S{j}", name=f"Oa{j}{sfx}"
                    )
                    nc.tensor.matmul(
                        Oa_ps[0:65, :],
                        vtsA[:, j * 65 : (j + 1) * 65],
                        P_j[:],
                        start=True,
                        stop=True,
                    )
                    Ob_ps = psp.tile(
                        [128, 512], fp32, tag=f"S{(j + 1) % NSLOT}", name=f"Ob{j}{sfx}"
                    )
                    nc.tensor.matmul(
                        Ob_ps[0:64, :],
                        vtsB[:, j * 64 : (j + 1) * 64],
                        P_j[:],
                        start=True,
                        stop=True,
                    )
                    nc.vector.tensor_scalar_mul(
                        oa_sb[:, j * 512 : (j + 1) * 512], Oa_ps[0:65, :], 1.0
                    )
                    nc.scalar.activation(
                        ob_sb[:, j * 512 : (j + 1) * 512], Ob_ps[0:64, :], Copy,
                        scale=1.0,
                    )
                    if rep == repeat - 1:
                        rings = (nc.sync, nc.scalar)
                        rings[(2 * j) % 2].dma_start(
                            outAE[:, j * 512 : (j + 1) * 512],
                            oa_sb[:, j * 512 : (j + 1) * 512],
                        )
                        rings[(2 * j + 1) % 2].dma_start(
                            outBE[:, j * 512 : (j + 1) * 512],
                            ob_sb[:, j * 512 : (j + 1) * 512],
                        )

    nc.compile()
    return nc


def _prepare(inputs):
    import ml_dtypes

    f16 = np.float16
    queries = np.asarray(inputs["queries"], dtype=np.float32)
    keys = np.asarray(inputs["keys"], dtype=np.float32)
    values = np.asarray(inputs["values"], dtype=np.float32)
    valid_lens = np.asarray(inputs["valid_lens"]).astype(np.int64)
    W_q = np.asarray(inputs["W_q"], dtype=np.float32)
    W_k = np.asarray(inputs["W_k"], dtype=np.float32)
    w_v = np.asarray(inputs["w_v"], dtype=np.float32)

    c1lin, bco = _fit_coeffs()
    cores = _plan([int(x) for x in valid_lens])

    wq2 = np.empty((128, 256), f16)
    wk2 = np.empty((128, 256), f16)
    for c in range(2):
        wq2[:, c * 128 : (c + 1) * 128] = np.tile(
            W_q[c * 128 : (c + 1) * 128] * OM1, (1, 2)
        ).astype(f16)
        wk2[:, c * 128 : (c + 1) * 128] = np.tile(
            W_k[c * 128 : (c + 1) * 128] * OM1, (1, 2)
        ).astype(f16)
    wvec = np.tile((0.5 * c1lin / OM1 * w_v)[:, None], (2, 1)).astype(f16)
    wbs = np.tile(w_v[:, None] * bco[None, :], (2, 1)).astype(np.float32)
    hp = np.zeros((128, 3), np.float32)
    hp[64:128, 0] = np.pi / 2
    hp[:, 1] = np.pi / 2
    hp[0:64, 2] = np.pi / 2

    qT = {b: np.ascontiguousarray(queries[b].T) for b in range(B)}

    in_maps = []
    for c in range(NCORES):
        jobs = cores[c]
        qts = np.zeros((128, 2048), f16)
        for s, j in ((0, 0), (1, 3)):
            b, qh, _, _ = jobs[j]
            for ch in range(2):
                qts[:, s * 1024 + ch * 512 : s * 1024 + (ch + 1) * 512] = qT[b][
                    ch * 128 : (ch + 1) * 128, qh * 512 : (qh + 1) * 512
                ].astype(f16)
        kts = np.zeros((128, 1024), f16)
        vtsA = np.zeros((128, NSLOT * 65), f16)
        vtsB = np.zeros((128, NSLOT * 64), f16)
        mask = np.full((128, 4), MASK_VAL, np.float32)
        for j, (b, qh, s0, cnt) in enumerate(jobs):
            vtsA[:, j * 65 + 64] = 1.0  # ones column -> l row
            if cnt == 0:
                continue
            kp = np.zeros((128, D_IN), np.float32)
            kp[0:cnt] = keys[b, s0 : s0 + cnt]
            kT = kp.T  # (256, 128)
            for ch in range(2):
                kts[:, ch * 512 + j * 128 : ch * 512 + (j + 1) * 128] = kT[
                    ch * 128 : (ch + 1) * 128
                ].astype(f16)
            vp = np.zeros((128, D_V), np.float32)
            vp[0:cnt] = values[b, s0 : s0 + cnt]
            vtsA[:, j * 65 : j * 65 + 64] = vp[:, 0:64].astype(f16)
            vtsB[:, j * 64 : (j + 1) * 64] = vp[:, 64:128].astype(f16)
            mask[0:cnt, j] = 0.0
        in_maps.append(
            {
                "qts": qts,
                "kts": kts,
                "vtsA": vtsA,
                "vtsB": vtsB,
                "mask": mask,
                "wq2": wq2,
                "wk2": wk2,
                "wvec": wvec,
                "wbs": wbs,
                "hp": hp,
            }
        )
    return cores, in_maps


def kernel(**inputs):
    global LAST_RESULT
    cores, in_maps = _prepare(inputs)

    if "nc" not in _CACHE:
        _CACHE["nc"] = _build()
    nc = _CACHE["nc"]

    from concourse.bass_utils import run_bass_kernel_spmd

    res = run_bass_kernel_spmd(nc, in_maps, core_ids=list(range(NCORES)))
    LAST_RESULT = res

    O = np.zeros((B, D_V, Q), np.float64)
    L = np.zeros((B, Q), np.float64)
    for c in range(NCORES):
        oA = np.asarray(res.results[c]["outA"]).astype(np.float64)
        oB = np.asarray(res.results[c]["outB"]).astype(np.float64)
        for j, (b, qh, s0, cnt) in enumerate(cores[c]):
            if cnt == 0:
                continue
            sl = slice(j * 512, (j + 1) * 512)
            qs = slice(qh * 512, (qh + 1) * 512)
            O[b][0:64, qs] += oA[0:64, sl]
            O[b][64:128, qs] += oB[:, sl]
            L[b][qs] += oA[64, sl]
    out = (O / L[:, None, :]).transpose(0, 2, 1)
    return np.ascontiguousarray(out.astype(np.float32))


# revision 53
# speedup vs baseline: 1.0689x; 1.0498x over previous
"""Additive (Bahdanau) attention on 8 TRN2 NeuronCores — separable-score
formulation.

Math per batch b:
    q = queries[b] @ W_q                  (Q, H)
    k = keys[b]    @ W_k                  (K, H)
    S[i, j] = sum_h w_v[h] * tanh(q[i,h] + k[j,h])
    out[b]  = softmax_j(S masked) @ values[b]

Instead of materializing the (Q, K, H) tanh tensor (elementwise-engine
bound), tanh is approximated by a regularized Fourier-extension series

    tanh(u) ~= c1*u + sum_{r=1..R} b_r sin(r*w1*u),   w1 = pi/PERIOD

whose angle-addition expansion is separable in (q, k):

    S[i,j] ~= sum_h sum_r (w_h b_r) [sin(r w1 q) cos(r w1 k)
                                     + cos(r w1 q) sin(r w1 k)]
              + c1 * sum_h w_h k_jh   (+ a per-row term softmax discards)

so S becomes R chained 128-contraction matmuls over trig features, and the
linear term folds into the exp bias.  The HW Sin table is only valid for
|arg| <= pi, so just sin/cos(w1 q) are computed on ACT (|w1 q| + pi/2 < pi)
and higher harmonics come from the Chebyshev angle ladder on DVE in fp16:

    X_r = 2 cos(w1 x) * X_{r-1} - X_{r-2}

which runs the sin and cos chains together in one (128, n) tile
(partitions = 64 h x {sin, cos}).  Rel err of the whole approximation
(incl. fp16 features) ~= 5e-3 vs the exact reference.

Sharding: the prefix mask means only sum(valid_lens) key columns carry
work.  Valid keys are split into <=128-key jobs; each of the 8 cores runs
4 job slots fed by two query feature sets (set0 serves slots 0-2, set1
serves slot 3; a set is one (batch, q-half) of 512 queries).  Each job
emits unnormalized partials (O^T = V^T P, l = 1^T P); the host sums
partials per batch and divides.
"""

import sys

sys.path.insert(0, "/opt/trn_rl_repo")

import numpy as np

B, Q, KLEN, D_IN, H, D_V = 4, 1024, 1024, 256, 64, 128
NCORES = 8
NSLOT = 4  # job slots per core: slots 0-2 -> set0, slot 3 -> set1
MASK_VAL = -1.0e6

R = 11  # Fourier harmonics
PERIOD = 13.0
OM1 = np.pi / PERIOD
UFIT = 9.8
LAM = 0.1

_CACHE = {}
_COEF = None
LAST_RESULT = None


def _fit_coeffs():
    """Host-side (weights-only) fit: tanh(u) ~= c1*u + sum b_r sin(r w1 u)."""
    global _COEF
    if _COEF is not None:
        return _COEF
    u = np.linspace(-UFIT, UFIT, 6001)
    w = 0.25 + np.exp(-(u**2) / (2 * 3.0**2))
    cols = [u / UFIT] + [np.sin((r + 1) * np.pi * u / PERIOD) for r in range(R)]
    A = np.stack(cols, axis=1)
    AtA = (A * w[:, None] ** 2).T @ A + LAM * np.eye(A.shape[1])
    Aty = (A * w[:, None] ** 2).T @ np.tanh(u)
    coef = np.linalg.solve(AtA, Aty)
    _COEF = (coef[0] / UFIT, coef[1:].copy())
    return _COEF


def _plan(vl):
    """Split each batch's valid-key prefix into <=128-key jobs per q-half and
    pack them into 8 cores x (one 3-job set0 + one 1-job set1).

    Returns cores: list of 8 entries, each a list of NSLOT jobs
    (b, qh, start, cnt) with cnt == 0 for padding slots."""
    combos = []  # ((b, qh), [(start, cnt), ...])
    for b, v in enumerate(vl):
        nb = -(-v // 128) if v > 0 else 0
        chunks = []
        s = 0
        for i in range(nb):
            cnt = min(128, v - s)
            chunks.append((s, cnt))
            s += cnt
        for qh in range(2):
            if chunks:
                combos.append(((b, qh), list(chunks)))
    assert sum(len(c[1]) for c in combos) <= 4 * NCORES, (
        "job count exceeds slot capacity; vl sum too large for this plan"
    )
    # Greedy: fill 8 triple-slots (set0) and 8 single-slots (set1).
    triples = []  # (combo, [jobs up to 3])
    singles = []  # (combo, job)
    combos = sorted(combos, key=lambda c: -len(c[1]))
    leftovers = []
    for key, chunks in combos:
        i = 0
        while len(chunks) - i >= 3 and len(triples) < NCORES:
            triples.append((key, chunks[i : i + 3]))
            i += 3
        leftovers.append((key, chunks[i:]))
    # place leftovers: prefer singles; group of 2-3 can take a triple slot
    for key, chunks in sorted(leftovers, key=lambda c: -len(c[1])):
        i = 0
        while len(chunks) - i >= 2 and len(triples) < NCORES:
            take = min(3, len(chunks) - i)
            triples.append((key, chunks[i : i + take]))
            i += take
        while i < len(chunks):
            if len(singles) < NCORES:
                singles.append((key, chunks[i]))
            elif len(triples) < NCORES:
                triples.append((key, [chunks[i]]))
            else:
                raise AssertionError("packing failed")
            i += 1
    while len(triples) < NCORES:
        triples.append(((0, 0), []))
    while len(singles) < NCORES:
        singles.append(((0, 0), None))
    cores = []
    for c in range(NCORES):
        (b0, qh0), t_jobs = triples[c]
        (b1, qh1), s_job = singles[c]
        jobs = []
        for i in range(3):
            if i < len(t_jobs):
                jobs.append((b0, qh0, t_jobs[i][0], t_jobs[i][1]))
            else:
                jobs.append((b0, qh0, 0, 0))
        if s_job is not None:
            jobs.append((b1, qh1, s_job[0], s_job[1]))
        else:
            jobs.append((b1, qh1, 0, 0))
        cores.append(jobs)
    return cores


def _build(repeat=1):
    import concourse.tile as tile
    from concourse import bacc, mybir

    fp32 = mybir.dt.float32
    fp16 = mybir.dt.float16
    Sin = mybir.ActivationFunctionType.Sin
    Exp = mybir.ActivationFunctionType.Exp
    Copy = mybir.ActivationFunctionType.Copy
    mult = mybir.AluOpType.mult
    sub = mybir.AluOpType.subtract
    add = mybir.AluOpType.add

    nc = bacc.Bacc(
        "TRN2", target_bir_lowering=False, debug=False, num_devices=NCORES
    )
    qtsE = nc.dram_tensor("qts", [128, 2048], fp16, kind="ExternalInput").ap()
    ktsE = nc.dram_tensor("kts", [128, 1024], fp16, kind="ExternalInput").ap()
    # vts: per job 65 cols [V[:, 0:64] | ones] (the ones column makes the
    # O-matmul emit l = 1^T P as output row 64), then per job 64 cols
    # V[:, 64:128].
    vtsE = nc.dram_tensor(
        "vts", [128, NSLOT * 65 + NSLOT * 64], fp16, kind="ExternalInput"
    ).ap()
    wq2E = nc.dram_tensor("wq2", [128, 256], fp16, kind="ExternalInput").ap()
    # wkx = [wk2 | wvec]: key projection stationary + beta moving vector
    wkxE = nc.dram_tensor("wkx", [128, 257], fp16, kind="ExternalInput").ap()
    # hpx = [hp(3) | wbs(R) | mask(4)] f32
    hpxE = nc.dram_tensor("hpx", [128, 7 + R], fp32, kind="ExternalInput").ap()
    bf16 = mybir.dt.bfloat16
    outAE = nc.dram_tensor("outA", [65, NSLOT * 512], bf16, kind="ExternalOutput").ap()
    outBE = nc.dram_tensor("outB", [64, NSLOT * 512], bf16, kind="ExternalOutput").ap()

    with tile.TileContext(nc) as tc:
        with (
            tc.tile_pool(name="const", bufs=1) as cp,
            tc.tile_pool(name="feat", bufs=2) as fp,
            tc.tile_pool(name="ps", bufs=1, space="PSUM") as psp,
        ):
            # ---- input DMAs (q-path first), spread across rings
            # Few, large input DMAs: each dma_start costs ~625ns on the shared
            # HWDGE queue regardless of size, so merge aggressively and lead
            # both rings with what the first projection needs.
            kts = cp.tile([128, 1024], fp16)
            nc.scalar.dma_start(kts[:, 0:512], ktsE[:, 0:512])
            wkx = cp.tile([128, 257], fp16)
            nc.sync.dma_start(wkx[:], wkxE[:, :])
            nc.sync.dma_start(kts[:, 512:1024], ktsE[:, 512:1024])
            wq2 = cp.tile([128, 256], fp16)
            nc.scalar.dma_start(wq2[:], wq2E[:, :])
            qts = cp.tile([128, 2048], fp16)
            nc.sync.dma_start(qts[:, 0:1024], qtsE[:, 0:1024])
            nc.scalar.dma_start(qts[:, 1024:2048], qtsE[:, 1024:2048])
            hpx = cp.tile([128, 7 + R], fp32)
            nc.gpsimd.dma_start(hpx[:], hpxE[:, :])
            vts = cp.tile([128, NSLOT * 65 + NSLOT * 64], fp16)
            nc.gpsimd.dma_start(vts[:], vtsE[:, :])
            VB0 = NSLOT * 65  # vtsB column base within vts
            MB = 3 + R  # mask column base within hpx

            # ladder r=0 tiles: q-side [sin0; cos0] = [0; 1], k-side
            # [cos0; sin0] (DVE memsets: DVE is idle until the anchors land)
            x0q = cp.tile([128, 1024], fp16, name="x0q")
            nc.vector.memset(x0q[0:64, :], 0.0)
            nc.vector.memset(x0q[64:128, :], 1.0)
            x0k = cp.tile([128, 512], fp16, name="x0k")
            nc.vector.memset(x0k[0:64, :], 1.0)
            nc.vector.memset(x0k[64:128, :], 0.0)

            # dummy 1-element activations: force the ACT table loads during
            # the input DMAs instead of on the anchor critical path (input =
            # the DVE-memset x0q tile, available at ~1us)
            with tc.high_priority():
                scrap = cp.tile([1, 1], fp16, name="scrap")
                nc.scalar.activation(scrap[:], x0q[0:1, 0:1], Sin)
                nc.scalar.activation(scrap[:], x0q[0:1, 0:1], Copy, scale=2.0)
                nc.scalar.activation(
                    scrap[:], x0q[0:1, 0:1], Sin, bias=1.0, scale=1.0
                )

            for rep in range(repeat):
                sfx = f"_{rep}"
                # ---- projections (stationaries duplicated so psum has both
                # halves: rows 0:64 and 64:128 hold identical h-values)
                kp_ps = psp.tile([128, 512], fp32, tag="kp", name=f"kp{sfx}")
                for c in range(2):
                    nc.tensor.matmul(
                        kp_ps[:],
                        wkx[:, c * 128 : (c + 1) * 128],
                        kts[:, c * 512 : (c + 1) * 512],
                        start=(c == 0),
                        stop=(c == 1),
                    )
                qp_ps = [
                    psp.tile([128, 512], fp32, tag=f"qp{s}", name=f"qp{s}{sfx}")
                    for s in range(2)
                ]
                for s in range(2):
                    for c in range(2):
                        nc.tensor.matmul(
                            qp_ps[s][:],
                            wq2[:, c * 128 : (c + 1) * 128],
                            qts[:, s * 1024 + c * 512 : s * 1024 + (c + 1) * 512],
                            start=(c == 0),
                            stop=(c == 1),
                        )

                # ---- ACT anchors (Sin table), k-side first: the k-chain
                # gates the bk copies and thus the Exp table load.
                xk = [fp.tile([128, 512], fp16, tag="xk", name=f"xk1{sfx}", bufs=2)]
                nc.scalar.activation(
                    xk[0][:], kp_ps[:], Sin, bias=hpx[:, 2:3], scale=1.0
                )
                ck1 = fp.tile([128, 512], fp16, tag="ck1", name=f"ck1{sfx}", bufs=2)
                nc.scalar.activation(ck1[:], kp_ps[:], Sin, bias=hpx[:, 1:2], scale=1.0)
                # kk = scaled keys for the beta matmul (the 1/OM1 unscaling is
                # folded into wvec host-side)
                kk = fp.tile([128, 512], fp16, tag="kk", name=f"kk{sfx}", bufs=2)
                nc.scalar.activation(kk[:], kp_ps[:], Copy, scale=1.0)
                dk = fp.tile([128, 512], fp16, tag="dk", name=f"dk{sfx}", bufs=2)
                nc.vector.tensor_scalar_mul(dk[:], ck1[:], 2.0)

                xq = [fp.tile([128, 1024], fp16, tag="xq", name=f"xq1{sfx}", bufs=2)]
                cq1 = fp.tile([128, 1024], fp16, tag="cq1", name=f"cq1{sfx}", bufs=2)
                for s in range(2):
                    nc.scalar.activation(
                        xq[0][:, s * 512 : (s + 1) * 512],
                        qp_ps[s][:],
                        Sin,
                        bias=hpx[:, 0:1],
                        scale=1.0,
                    )
                    nc.scalar.activation(
                        cq1[:, s * 512 : (s + 1) * 512],
                        qp_ps[s][:],
                        Sin,
                        bias=hpx[:, 1:2],
                        scale=1.0,
                    )

                # doubled-cos multiplier tile (q side)
                dq = fp.tile([128, 1024], fp16, tag="dq", name=f"dq{sfx}", bufs=2)
                nc.vector.tensor_scalar_mul(dq[:], cq1[:], 2.0)

                # ---- scaled k-features for r=1 (ACT, Sin-table Copy)
                bk = [fp.tile([128, 512], fp16, tag="bk1", name=f"bk1{sfx}", bufs=2)]
                nc.scalar.activation(bk[0][:], xk[0][:], Copy, scale=hpx[:, 3:4])

                # ---- beta matmuls (linear-term bias per key)
                beta_ps = psp.tile([128, 4], fp32, tag="beta", name=f"beta{sfx}")
                for j in range(NSLOT):
                    nc.tensor.matmul(
                        beta_ps[:, j : j + 1],
                        kk[:, j * 128 : (j + 1) * 128],
                        wkx[:, 256:257],
                        start=True,
                        stop=True,
                    )
                betam = fp.tile([128, 4], fp32, tag="betam", name=f"betam{sfx}", bufs=2)
                for j in range(NSLOT):
                    nc.vector.tensor_tensor(
                        betam[:, j : j + 1],
                        beta_ps[:, j : j + 1],
                        hpx[:, MB + j : MB + j + 1],
                        add,
                    )

                # ---- ladders (DVE) + scaled copies (ACT) + S passes (PE)
                # PSUM bank budget is 8: qp0, qp1, kp, beta + S0-3.  The Oa/Ob
                # output accumulators cycle through the S banks once each S is
                # consumed by its exp, so next rep's projections never wait on
                # this rep's tail.
                S_ps = [
                    psp.tile([128, 512], fp32, tag=f"S{j}", name=f"S{j}{sfx}")
                    for j in range(NSLOT)
                ]
                setof = (0, 0, 0, 1)
                for j in range(NSLOT):
                    s = setof[j]
                    nc.tensor.matmul(
                        S_ps[j][:],
                        bk[0][:, j * 128 : (j + 1) * 128],
                        xq[0][:, s * 512 : (s + 1) * 512],
                        start=True,
                        stop=False,
                    )
                # k-side ladder first (high priority so the scheduler finishes
                # the k-chain early: the last bk copy gates the Exp table
                # load, which otherwise lands on the critical tail).
                KSPLIT = R
                with tc.high_priority():
                    for r in range(2, R + 1):
                        keng = nc.vector if r <= KSPLIT else nc.gpsimd
                        tk = fp.tile(
                            [128, 512], fp16, tag="tk", name=f"tk{r}{sfx}", bufs=2
                        )
                        keng.tensor_tensor(tk[:], dk[:], xk[-1][:], mult)
                        xk.append(
                            fp.tile(
                                [128, 512], fp16, tag=f"xk{r}", name=f"xk{r}{sfx}", bufs=2
                            )
                        )
                        prev2k = x0k if r == 2 else xk[-3]
                        keng.tensor_tensor(xk[-1][:], tk[:], prev2k[:], sub)
                        # scaled k-features
                        bk.append(
                            fp.tile(
                                [128, 512], fp16, tag=f"bk{r}", name=f"bk{r}{sfx}", bufs=2
                            )
                        )
                        nc.scalar.activation(
                            bk[-1][:], xk[-1][:], Copy, scale=hpx[:, 2 + r : 3 + r]
                        )
                QSPL = 768  # q-ladder col split: [0:QSPL] DVE, [QSPL:] Pool
                for r in range(2, R + 1):
                    # q-side ladder step, column-split across DVE and Pool
                    tq = fp.tile(
                        [128, 1024], fp16, tag="tq", name=f"tq{r}{sfx}", bufs=2
                    )
                    nc.vector.tensor_tensor(
                        tq[:, 0:QSPL], dq[:, 0:QSPL], xq[-1][:, 0:QSPL], mult
                    )
                    nc.gpsimd.tensor_tensor(
                        tq[:, QSPL:1024], dq[:, QSPL:1024], xq[-1][:, QSPL:1024], mult
                    )
                    xq.append(
                        fp.tile([128, 1024], fp16, tag=f"xq{r % 3}", name=f"xq{r}{sfx}", bufs=2)
                    )
                    prev2q = x0q if r == 2 else xq[-3]
                    nc.vector.tensor_tensor(
                        xq[-1][:, 0:QSPL], tq[:, 0:QSPL], prev2q[:, 0:QSPL], sub
                    )
                    nc.gpsimd.tensor_tensor(
                        xq[-1][:, QSPL:1024], tq[:, QSPL:1024], prev2q[:, QSPL:1024], sub
                    )
                    # S passes for all four jobs at this harmonic
                    for j in range(NSLOT):
                        s = setof[j]
                        nc.tensor.matmul(
                            S_ps[j][:],
                            bk[r - 1][:, j * 128 : (j + 1) * 128],
                            xq[-1][:, s * 512 : (s + 1) * 512],
                            start=False,
                            stop=(r == R),
                        )

                # ---- softmax numerator + weighted values (l rides as row 64
                # of the A-half O matmul via the ones column in vtsA)
                oa_sb = fp.tile(
                    [65, NSLOT * 512], bf16, tag="oasb", name=f"oa{sfx}", bufs=1
                )
                ob_sb = fp.tile(
                    [64, NSLOT * 512], bf16, tag="obsb", name=f"ob{sfx}", bufs=1
                )
                for j in range(NSLOT):
                    P_j = fp.tile(
                        [128, 512], fp16, tag=f"P{j % 2}", name=f"P{j}{sfx}", bufs=2
                    )
                    nc.scalar.activation(
                        P_j[:], S_ps[j][:], Exp, bias=betam[:, j : j + 1], scale=1.0
                    )
                    Oa_ps = psp.tile(
                        [128, 512], fp32, tag=f"S{j}", name=f"Oa{j}{sfx}"
                    )
                    nc.tensor.matmul(
                        Oa_ps[0:65, :],
                        vts[:, j * 65 : (j + 1) * 65],
                        P_j[:],
                        start=True,
                        stop=True,
                    )
                    Ob_ps = psp.tile(
                        [128, 512], fp32, tag=f"S{(j + 1) % NSLOT}", name=f"Ob{j}{sfx}"
                    )
                    nc.tensor.matmul(
                        Ob_ps[0:64, :],
                        vts[:, VB0 + j * 64 : VB0 + (j + 1) * 64],
                        P_j[:],
                        start=True,
                        stop=True,
                    )
                    nc.vector.tensor_scalar_mul(
                        oa_sb[:, j * 512 : (j + 1) * 512], Oa_ps[0:65, :], 1.0
                    )
                    nc.scalar.activation(
                        ob_sb[:, j * 512 : (j + 1) * 512], Ob_ps[0:64, :], Copy,
                        scale=1.0,
                    )
                    if rep == repeat - 1:
                        rings = (nc.sync, nc.scalar)
                        rings[(2 * j) % 2].dma_start(
                            outAE[:, j * 512 : (j + 1) * 512],
                            oa_sb[:, j * 512 : (j + 1) * 512],
                        )
                        rings[(2 * j + 1) % 2].dma_start(
                            outBE[:, j * 512 : (j + 1) * 512],
                            ob_sb[:, j * 512 : (j + 1) * 512],
                        )

    nc.compile()
    return nc


def _prepare(inputs):
    import ml_dtypes

    f16 = np.float16
    queries = np.asarray(inputs["queries"], dtype=np.float32)
    keys = np.asarray(inputs["keys"], dtype=np.float32)
    values = np.asarray(inputs["values"], dtype=np.float32)
    valid_lens = np.asarray(inputs["valid_lens"]).astype(np.int64)
    W_q = np.asarray(inputs["W_q"], dtype=np.float32)
    W_k = np.asarray(inputs["W_k"], dtype=np.float32)
    w_v = np.asarray(inputs["w_v"], dtype=np.float32)

    c1lin, bco = _fit_coeffs()
    cores = _plan([int(x) for x in valid_lens])

    wq2 = np.empty((128, 256), f16)
    wk2 = np.empty((128, 256), f16)
    for c in range(2):
        wq2[:, c * 128 : (c + 1) * 128] = np.tile(
            W_q[c * 128 : (c + 1) * 128] * OM1, (1, 2)
        ).astype(f16)
        wk2[:, c * 128 : (c + 1) * 128] = np.tile(
            W_k[c * 128 : (c + 1) * 128] * OM1, (1, 2)
        ).astype(f16)
    wkx = np.empty((128, 257), f16)
    wkx[:, 0:256] = wk2
    wkx[:, 256] = np.tile(0.5 * c1lin / OM1 * w_v, 2).astype(f16)
    hpx0 = np.zeros((128, 7 + R), np.float32)
    hpx0[64:128, 0] = np.pi / 2
    hpx0[:, 1] = np.pi / 2
    hpx0[0:64, 2] = np.pi / 2
    hpx0[:, 3 : 3 + R] = np.tile(w_v[:, None] * bco[None, :], (2, 1))

    qT = {b: np.ascontiguousarray(queries[b].T) for b in range(B)}

    in_maps = []
    for c in range(NCORES):
        jobs = cores[c]
        qts = np.zeros((128, 2048), f16)
        for s, j in ((0, 0), (1, 3)):
            b, qh, _, _ = jobs[j]
            for ch in range(2):
                qts[:, s * 1024 + ch * 512 : s * 1024 + (ch + 1) * 512] = qT[b][
                    ch * 128 : (ch + 1) * 128, qh * 512 : (qh + 1) * 512
                ].astype(f16)
        kts = np.zeros((128, 1024), f16)
        VB0 = NSLOT * 65
        vts = np.zeros((128, NSLOT * 65 + NSLOT * 64), f16)
        hpx = hpx0.copy()
        hpx[:, 3 + R : 7 + R] = MASK_VAL
        for j, (b, qh, s0, cnt) in enumerate(jobs):
            vts[:, j * 65 + 64] = 1.0  # ones column -> l row
            if cnt == 0:
                continue
            kp = np.zeros((128, D_IN), np.float32)
            kp[0:cnt] = keys[b, s0 : s0 + cnt]
            kT = kp.T  # (256, 128)
            for ch in range(2):
                kts[:, ch * 512 + j * 128 : ch * 512 + (j + 1) * 128] = kT[
                    ch * 128 : (ch + 1) * 128
                ].astype(f16)
            vp = np.zeros((128, D_V), np.float32)
            vp[0:cnt] = values[b, s0 : s0 + cnt]
            vts[:, j * 65 : j * 65 + 64] = vp[:, 0:64].astype(f16)
            vts[:, VB0 + j * 64 : VB0 + (j + 1) * 64] = vp[:, 64:128].astype(f16)
            hpx[0:cnt, 3 + R + j] = 0.0
        in_maps.append(
            {
                "qts": qts,
                "kts": kts,
                "vts": vts,
                "wq2": wq2,
                "wkx": wkx,
                "hpx": hpx,
            }
        )
    return cores, in_maps


def kernel(**inputs):
    global LAST_RESULT
    cores, in_maps = _prepare(inputs)

    if "nc" not in _CACHE:
        _CACHE["nc"] = _build()
    nc = _CACHE["nc"]

    from concourse.bass_utils import run_bass_kernel_spmd

    res = run_bass_kernel_spmd(nc, in_maps, core_ids=list(range(NCORES)))
    LAST_RESULT = res

    O = np.zeros((B, D_V, Q), np.float64)
    L = np.zeros((B, Q), np.float64)
    for c in range(NCORES):
        oA = np.asarray(res.results[c]["outA"]).astype(np.float64)
        oB = np.asarray(res.results[c]["outB"]).astype(np.float64)
        for j, (b, qh, s0, cnt) in enumerate(cores[c]):
            if cnt == 0:
                continue
            sl = slice(j * 512, (j + 1) * 512)
            qs = slice(qh * 512, (qh + 1) * 512)
            O[b][0:64, qs] += oA[0:64, sl]
            O[b][64:128, qs] += oB[:, sl]
            L[b][qs] += oA[64, sl]
    out = (O / L[:, None, :]).transpose(0, 2, 1)
    return np.ascontiguousarray(out.astype(np.float32))
